# revision 43
# baseline (speedup 1.0000x reference)
"""DCN cross-layer stack on 8 Trainium2 NeuronCores (data parallel over batch).

Math: the cross layer x_{l+1} = x_0 * (x_l @ W_i) + b_i + bias_i + x_l keeps
x_l in the form  x_l = x_0 * alpha_l + gamma_l  with alpha_l a per-row scalar
and gamma_l a constant row vector:
    p_i  = x_0 @ W_i                  (per-row, on device)
    q_i  = gamma_i . W_i              (scalar, host — parameter-only)
    alpha_{i+1} = alpha_i*(1+p_i) + q_i
    gamma_{i+1} = gamma_i + (b_i + bias_i)
    out = x_0 * alpha_L + gamma_L

v11: fp16 wire format (gate 2e-2; fp16 end-to-end sims at ~5e-4).  Each
issuing engine owns one HW DMA queue (~90-160 GB/s each), so the 1.5 MB of
traffic is spread over all three queues.  The weight image rides at the head
of the xT_A transfer (tiny standalone DMAs waste ~1 us in RMW descriptors).
P on PE (fp16 single pass), FD=4 DVE recurrence per half, combines on DVE
with one tile on ACT, quarter-sized fp16 outputs that the host upcasts.
"""

import os
from contextlib import ExitStack

import numpy as np

import concourse.bacc as bacc
import concourse.bass as bass
import concourse.tile as tile
from concourse import mybir
from concourse.bass_utils import run_bass_kernel_spmd

FP32 = mybir.dt.float32
FP16 = mybir.dt.float16

B_FULL = 8192
D = 256
L = 4
N_CORES = 8
B_CORE = B_FULL // N_CORES  # 1024
NT = B_CORE // 128  # 8 row-tiles per core
NH = 2

_cache = {}
last_exec_time_ns = None
last_results = None


def _build_nc(q, zero_gamma):
    """q: tuple of L python floats (q_i). zero_gamma: skip the +gamma add."""
    nc = bacc.Bacc(
        "TRN2", target_bir_lowering=False, debug=False, num_devices=N_CORES,
        num_swdge_queues=2,
    )
    # xa16[p, 0:8]  = wt (wt[p, 4h+l] = W[l, 128h+p])
    # xa16[p, 8+c*512+h*256+j] = x[256c + j, 128h + p]       (pieces 0-1)
    xa_in = nc.declare_dram_parameter("xa16", [128, 8 + 1024], FP16, isOutput=False)
    # xb16[p, c*512+h*256+j]   = x[256(c+2) + j, 128h + p]   (pieces 2-3)
    xb_in = nc.declare_dram_parameter("xb16", [128, 1024], FP16, isOutput=False)
    # xd[p, t, d] = x[128t + p, d]
    x_in = nc.declare_dram_parameter("x16", [128, NT, D], FP16, isOutput=False)
    if not zero_gamma:
        gb_in = nc.declare_dram_parameter("gammab", [128, D], FP32, isOutput=False)
    out_ext = nc.declare_dram_parameter("out16", [128, NT, D], FP16, isOutput=True)

    with tile.TileContext(nc) as tc, ExitStack() as ctx:
        xtp = ctx.enter_context(tc.tile_pool(name="xtp", bufs=1))
        xin = ctx.enter_context(tc.tile_pool(name="xin", bufs=1))
        pps = ctx.enter_context(
            tc.tile_pool(name="pps", bufs=1, space=bass.MemorySpace.PSUM)
        )
        apool = ctx.enter_context(tc.tile_pool(name="apool", bufs=1))
        outp = ctx.enter_context(tc.tile_pool(name="outp", bufs=1))
        consts = ctx.enter_context(tc.tile_pool(name="consts", bufs=1))

        # one transfer per HWDGE ring for the transposed stream (+weights)
        xa = xtp.tile([128, 8 + 1024], FP16)
        nc.sync.dma_start(out=xa[:], in_=xa_in[:, :])
        xb = xtp.tile([128, 1024], FP16)
        nc.scalar.dma_start(out=xb[:], in_=xb_in[:, :])
        if not zero_gamma:
            gb = consts.tile([128, D], FP32)
            nc.gpsimd.dma_start(out=gb[:], in_=gb_in[:, :])

        # natural-x quarters: two on the gpsimd queue, two on the ring tails
        x_t = []
        x_engs = (nc.gpsimd, nc.gpsimd, nc.gpsimd, nc.gpsimd)
        for c in range(4):
            xh = xin.tile([128, 2, D], FP16, tag=f"x{c}")
            x_engs[c].dma_start(out=xh[:], in_=x_in[:, 2 * c : 2 * c + 2, :])
            x_t.append(xh)

        def lhsT(c, h, s):  # xT block [128, 128] for piece c, half h, subtile s
            src, base = (xa, 8) if c < 2 else (xb, 0)
            off = base + (c % 2) * 512 + h * 256 + 128 * s
            return src[:, off : off + 128]

        def wT(h):
            return xa[:, 4 * h : 4 * h + 4]

        # P matmuls: pieces 2g, 2g+1 share one PSUM tensor so the recurrence
        # covers a whole half in FD=4 ops
        P_h = []
        for g in range(NH):
            P_g = pps.tile([128, 2, 2, L], FP32, tag=f"P{g}")
            P_h.append(P_g)
        for c in range(4):
            g, ci = divmod(c, 2)
            for tt in range(2):
                nc.tensor.matmul(
                    P_h[g][:, ci, tt, :], lhsT(c, 0, tt), wT(0),
                    start=True, stop=False,
                )
                nc.tensor.matmul(
                    P_h[g][:, ci, tt, :], lhsT(c, 1, tt), wT(1),
                    start=False, stop=True,
                )

        # alpha recurrence on DVE: a_i = (P_i + 1) * a_{i-1} (+ q_i)
        alphas = [None] * 4  # fp32 [128, 2, 1] per piece
        for g in range(NH):
            a = apool.tile([128, 2, 2, L], FP32, tag=f"a{g}")
            nc.vector.tensor_scalar_add(
                a[:, :, :, 0], P_h[g][:, :, :, 0], 1.0 + q[0]
            )
            src = a[:, :, :, 0]
            for i in range(1, L):
                nc.vector.scalar_tensor_tensor(
                    a[:, :, :, i],
                    P_h[g][:, :, :, i],
                    1.0,
                    src,
                    op0=mybir.AluOpType.add,
                    op1=mybir.AluOpType.mult,
                )
                if q[i] != 0.0:
                    nc.vector.tensor_scalar_add(
                        a[:, :, :, i], a[:, :, :, i], q[i]
                    )
                src = a[:, :, :, i]
            alphas[2 * g] = a[:, 0, :, L - 1 : L]
            alphas[2 * g + 1] = a[:, 1, :, L - 1 : L]

        # combine + quarter-sized store per piece; tile 3 on ACT, rest DVE
        for c in range(4):
            o_c = outp.tile([128, 2, D], FP16, tag=f"o{c}")
            for tt in range(2):
                alpha_col = alphas[c][:, tt, 0:1]
                x_src = x_t[c][:, tt, :]
                if zero_gamma:
                    if (c, tt) == (1, 1):
                        nc.scalar.activation(
                            o_c[:, tt, :],
                            x_src,
                            mybir.ActivationFunctionType.Copy,
                            bias=0.0,
                            scale=alpha_col,
                        )
                    else:
                        nc.vector.tensor_scalar_mul(o_c[:, tt, :], x_src, alpha_col)
                else:
                    tmp = outp.tile([128, D], FP32, tag="tmp")
                    nc.vector.tensor_scalar_mul(tmp[:], x_src, alpha_col)
                    nc.vector.tensor_add(o_c[:, tt, :], tmp[:], gb[:])
            oeng = (nc.gpsimd, nc.scalar, nc.scalar, nc.sync)[c]
            oeng.dma_start(
                out=out_ext[:, 2 * c : 2 * c + 2, :], in_=o_c[:]
            )
    nc.finalize()
    return nc


def kernel(x, W, b_lin, bias):
    global last_exec_time_ns, last_results
    x = np.ascontiguousarray(x, dtype=np.float32)
    W = np.asarray(W, dtype=np.float32)
    b_lin = np.asarray(b_lin, dtype=np.float32)
    bias = np.asarray(bias, dtype=np.float32)

    # host-side exact collapse of the bias terms (parameter-only precompute)
    c = b_lin[:, None].astype(np.float64) + bias.astype(np.float64)  # [L, D]
    Wd = W.astype(np.float64)
    gamma = np.zeros(D, dtype=np.float64)
    q = np.zeros(L, dtype=np.float64)
    for i in range(L):
        q[i] = float(gamma @ Wd[i])
        gamma = gamma + c[i]
    zero_gamma = not np.any(gamma) and not np.any(q)
    q_f = tuple(float(np.float32(v)) for v in q)

    key = (q_f, zero_gamma)
    if key not in _cache:
        _cache[key] = _build_nc(q_f, zero_gamma)
    nc = _cache[key]

    wt16 = np.ascontiguousarray(
        W.astype(np.float16).reshape(L, 2, 128).transpose(2, 1, 0)
    ).reshape(128, 8)  # wt[p, 4h+l] = W[l, 128h+p]
    in_maps = []
    for core in range(N_CORES):
        xs16 = x[core * B_CORE : (core + 1) * B_CORE].astype(np.float16)
        # xT[p, c, h, j] = x[256c + j, 128h + p]
        xT = xs16.reshape(4, 256, 2, 128).transpose(3, 0, 2, 1)
        m = {
            "xa16": np.ascontiguousarray(
                np.concatenate([wt16, xT[:, 0:2].reshape(128, 1024)], axis=1)
            ),
            "xb16": np.ascontiguousarray(xT[:, 2:4].reshape(128, 1024)),
            # x16[p, t, d] = x[128t+p, d]
            "x16": np.ascontiguousarray(
                xs16.reshape(NT, 128, D).transpose(1, 0, 2)
            ),
        }
        if not zero_gamma:
            m["gammab"] = np.broadcast_to(
                gamma.astype(np.float32), (128, D)
            ).copy()
        in_maps.append(m)

    trace = bool(os.environ.get("KERNEL_TRACE"))
    res = run_bass_kernel_spmd(nc, in_maps, list(range(N_CORES)), trace=trace)
    last_exec_time_ns = res.exec_time_ns
    last_results = res
    out = np.concatenate(
        [
            r["out16"].transpose(1, 0, 2).reshape(B_CORE, D).astype(np.float32)
            for r in res.results
        ],
        axis=0,
    )
    return out


# revision 46
# speedup vs baseline: 1.1073x; 1.1073x over previous
"""DCN cross-layer stack on 8 Trainium2 NeuronCores (data parallel over batch).

Math: the cross layer x_{l+1} = x_0 * (x_l @ W_i) + b_i + bias_i + x_l keeps
x_l in the form  x_l = x_0 * alpha_l + gamma_l  with alpha_l a per-row scalar
and gamma_l a constant row vector:
    p_i  = x_0 @ W_i                  (per-row, on device)
    q_i  = gamma_i . W_i              (scalar, host — parameter-only)
    alpha_{i+1} = alpha_i*(1+p_i) + q_i
    gamma_{i+1} = gamma_i + (b_i + bias_i)
    out = x_0 * alpha_L + gamma_L

v11: fp16 wire format (gate 2e-2; fp16 end-to-end sims at ~5e-4).  Each
issuing engine owns one HW DMA queue (~90-160 GB/s each), so the 1.5 MB of
traffic is spread over all three queues.  The weight image rides at the head
of the xT_A transfer (tiny standalone DMAs waste ~1 us in RMW descriptors).
P on PE (fp16 single pass), FD=4 DVE recurrence per half, combines on DVE
with one tile on ACT, quarter-sized fp16 outputs that the host upcasts.
"""

import os
from contextlib import ExitStack

import numpy as np

import concourse.bacc as bacc
import concourse.bass as bass
import concourse.tile as tile
from concourse import mybir
from concourse.bass_utils import run_bass_kernel_spmd

FP32 = mybir.dt.float32
FP16 = mybir.dt.float16

B_FULL = 8192
D = 256
L = 4
N_CORES = 8
B_CORE = B_FULL // N_CORES  # 1024
NT = B_CORE // 128  # 8 row-tiles per core
NH = 2

_cache = {}
last_exec_time_ns = None
last_results = None


def _build_nc(q, zero_gamma):
    """q: tuple of L python floats (q_i). zero_gamma: skip the +gamma add."""
    nc = bacc.Bacc(
        "TRN2", target_bir_lowering=False, debug=False, num_devices=N_CORES,
        num_swdge_queues=2,
    )
    # xa16[p, 0:8]  = wt (wt[p, 4h+l] = W[l, 128h+p])
    # xa16[p, 8+c*512+h*256+j] = x[256c + j, 128h + p]       (pieces 0-1)
    xa_in = nc.declare_dram_parameter("xa16", [128, 8 + 1024], FP16, isOutput=False)
    # xb16[p, c*512+h*256+j]   = x[256(c+2) + j, 128h + p]   (pieces 2-3)
    xb_in = nc.declare_dram_parameter("xb16", [128, 1024], FP16, isOutput=False)
    # xd[p, t, d] = x[128t + p, d]
    x_in = nc.declare_dram_parameter("x16", [128, NT, D], FP16, isOutput=False)
    if not zero_gamma:
        gb_in = nc.declare_dram_parameter("gammab", [128, D], FP32, isOutput=False)
    out_ext = nc.declare_dram_parameter("out16", [128, NT, D], FP16, isOutput=True)

    with tile.TileContext(nc) as tc, ExitStack() as ctx:
        xtp = ctx.enter_context(tc.tile_pool(name="xtp", bufs=1))
        xin = ctx.enter_context(tc.tile_pool(name="xin", bufs=1))
        pps = ctx.enter_context(
            tc.tile_pool(name="pps", bufs=1, space=bass.MemorySpace.PSUM)
        )
        apool = ctx.enter_context(tc.tile_pool(name="apool", bufs=1))
        outp = ctx.enter_context(tc.tile_pool(name="outp", bufs=1))
        consts = ctx.enter_context(tc.tile_pool(name="consts", bufs=1))

        # one transfer per HWDGE ring for the transposed stream (+weights)
        xa = xtp.tile([128, 8 + 1024], FP16)
        nc.sync.dma_start(out=xa[:], in_=xa_in[:, :])
        xb = xtp.tile([128, 1024], FP16)
        nc.scalar.dma_start(out=xb[:], in_=xb_in[:, :])
        if not zero_gamma:
            gb = consts.tile([128, D], FP32)
            nc.gpsimd.dma_start(out=gb[:], in_=gb_in[:, :])

        # natural-x quarters: two on the gpsimd queue, two on the ring tails
        x_t = []
        x_engs = (nc.gpsimd, nc.gpsimd, nc.gpsimd, nc.gpsimd)
        for c in range(4):
            xh = xin.tile([128, 2, D], FP16, tag=f"x{c}")
            x_engs[c].dma_start(out=xh[:], in_=x_in[:, 2 * c : 2 * c + 2, :])
            x_t.append(xh)

        def lhsT(c, h, s):  # xT block [128, 128] for piece c, half h, subtile s
            src, base = (xa, 8) if c < 2 else (xb, 0)
            off = base + (c % 2) * 512 + h * 256 + 128 * s
            return src[:, off : off + 128]

        def wT(h):
            return xa[:, 4 * h : 4 * h + 4]

        # P matmuls: pieces 2g, 2g+1 share one PSUM tensor so the recurrence
        # covers a whole half in FD=4 ops
        P_h = []
        for g in range(NH):
            P_g = pps.tile([128, 2, 2, L], FP32, tag=f"P{g}")
            P_h.append(P_g)
        for c in range(4):
            g, ci = divmod(c, 2)
            for tt in range(2):
                nc.tensor.matmul(
                    P_h[g][:, ci, tt, :], lhsT(c, 0, tt), wT(0),
                    start=True, stop=False,
                )
                nc.tensor.matmul(
                    P_h[g][:, ci, tt, :], lhsT(c, 1, tt), wT(1),
                    start=False, stop=True,
                )

        # alpha recurrence on DVE: a_i = (P_i + 1) * a_{i-1} (+ q_i)
        alphas = [None] * 4  # fp32 [128, 2, 1] per piece
        for g in range(NH):
            a = apool.tile([128, 2, 2, L], FP32, tag=f"a{g}")
            nc.vector.tensor_scalar_add(
                a[:, :, :, 0], P_h[g][:, :, :, 0], 1.0 + q[0]
            )
            src = a[:, :, :, 0]
            for i in range(1, L):
                nc.vector.scalar_tensor_tensor(
                    a[:, :, :, i],
                    P_h[g][:, :, :, i],
                    1.0,
                    src,
                    op0=mybir.AluOpType.add,
                    op1=mybir.AluOpType.mult,
                )
                if q[i] != 0.0:
                    nc.vector.tensor_scalar_add(
                        a[:, :, :, i], a[:, :, :, i], q[i]
                    )
                src = a[:, :, :, i]
            alphas[2 * g] = a[:, 0, :, L - 1 : L]
            alphas[2 * g + 1] = a[:, 1, :, L - 1 : L]

        # combine + quarter-sized store per piece; tile 3 on ACT, rest DVE
        for c in range(4):
            o_c = outp.tile([128, 2, D], FP16, tag=f"o{c}")
            for tt in range(2):
                alpha_col = alphas[c][:, tt, 0:1]
                x_src = x_t[c][:, tt, :]
                if zero_gamma:
                    if (c, tt) == (1, 1):
                        nc.scalar.activation(
                            o_c[:, tt, :],
                            x_src,
                            mybir.ActivationFunctionType.Copy,
                            bias=0.0,
                            scale=alpha_col,
                        )
                    else:
                        nc.vector.tensor_scalar_mul(o_c[:, tt, :], x_src, alpha_col)
                else:
                    tmp = outp.tile([128, D], FP32, tag="tmp")
                    nc.vector.tensor_scalar_mul(tmp[:], x_src, alpha_col)
                    nc.vector.tensor_add(o_c[:, tt, :], tmp[:], gb[:])
            oeng = (nc.gpsimd, nc.scalar, nc.scalar, nc.sync)[c]
            oeng.dma_start(
                out=out_ext[:, 2 * c : 2 * c + 2, :], in_=o_c[:]
            )
    nc.finalize()
    return nc


def kernel(x, W, b_lin, bias):
    global last_exec_time_ns, last_results
    x = np.ascontiguousarray(x, dtype=np.float32)
    W = np.asarray(W, dtype=np.float32)
    b_lin = np.asarray(b_lin, dtype=np.float32)
    bias = np.asarray(bias, dtype=np.float32)

    # host-side exact collapse of the bias terms (parameter-only precompute)
    c = b_lin[:, None].astype(np.float64) + bias.astype(np.float64)  # [L, D]
    Wd = W.astype(np.float64)
    gamma = np.zeros(D, dtype=np.float64)
    q = np.zeros(L, dtype=np.float64)
    for i in range(L):
        q[i] = float(gamma @ Wd[i])
        gamma = gamma + c[i]
    zero_gamma = not np.any(gamma) and not np.any(q)
    q_f = tuple(float(np.float32(v)) for v in q)

    key = (q_f, zero_gamma)
    if key not in _cache:
        _cache[key] = _build_nc(q_f, zero_gamma)
    nc = _cache[key]

    wt16 = np.ascontiguousarray(
        W.astype(np.float16).reshape(L, 2, 128).transpose(2, 1, 0)
    ).reshape(128, 8)  # wt[p, 4h+l] = W[l, 128h+p]
    in_maps = []
    for core in range(N_CORES):
        xs16 = x[core * B_CORE : (core + 1) * B_CORE].astype(np.float16)
        # xT[p, c, h, j] = x[256c + j, 128h + p]
        xT = xs16.reshape(4, 256, 2, 128).transpose(3, 0, 2, 1)
        m = {
            "xa16": np.ascontiguousarray(
                np.concatenate([wt16, xT[:, 0:2].reshape(128, 1024)], axis=1)
            ),
            "xb16": np.ascontiguousarray(xT[:, 2:4].reshape(128, 1024)),
            # x16[p, t, d] = x[128t+p, d]
            "x16": np.ascontiguousarray(
                xs16.reshape(NT, 128, D).transpose(1, 0, 2)
            ),
        }
        if not zero_gamma:
            m["gammab"] = np.broadcast_to(
                gamma.astype(np.float32), (128, D)
            ).copy()
        in_maps.append(m)

    trace = bool(os.environ.get("KERNEL_TRACE"))
    res = run_bass_kernel_spmd(nc, in_maps, list(range(N_CORES)), trace=trace)
    last_exec_time_ns = res.exec_time_ns
    last_results = res
    out = np.concatenate(
        [
            r["out16"].transpose(1, 0, 2).reshape(B_CORE, D).astype(np.float32)
            for r in res.results
        ],
        axis=0,
    )
    return out
